# revision 51
# baseline (speedup 1.0000x reference)
"""Trainium2 Bass kernel for nn_AssignAttention (softmax over the query axis).

Math (per batch b):
  q = (query @ Wq)  [N, C] -> heads [N, H, hd]
  k = (key   @ Wk)  [S, C] -> heads [S, H, hd]
  raw[h, n, s] = (q_h @ k_h^T) * hd^-0.5
  attn = softmax(raw, axis=n)                  # normalize over queries, per (h, s)
  attn = attn / max(sum_s attn, 1)             # clamp-normalize over s, per (h, n)
  out[n, h*hd:  ] = sum_s attn[h, n, s] * key[s, h*hd: (h+1)*hd]
  returns (out, out_style) with out_style == out

Distribution: data-parallel over B=16 across 8 NeuronCores (2 batches/core).

v5 structure:
  - host pre-transposes/casts key/query to bf16 (no PE transposes on chip).
  - k-projection is FUSED into the attention tile loop (one m-chunk per
    tile) so the PE stream is dense (HAM stays warm) and there is no
    serial per-batch projection stage.
  - score sets are 1 PSUM bank (2 same-parity heads) on a 3-bank rotation;
    acts are [128, 512], 4 per tile.  Head h lives at et slot
    E=[0,2,1,3,4,6,5,7][h].
  - out matmuls: stationary = [v_h/D | 1/D] (65 cols), moving = e_h; PSUM
    row 64 accumulates div.  out(t) is issued one iteration late so ready
    work always fills the tensor queue while acts run.
  - den via 2x-mode DVE fold chain + short 1x reduce; vaug on GpSimd.
"""

import threading

import numpy as np

B, N, S, C, H = 16, 256, 4096, 512, 8
HD = C // H
NCORES = 8
BL = B // NCORES  # batches per core
SCALE = float(HD) ** -0.5

_cache = {}
_lock = threading.Lock()

# head -> et slot (set*2 + pos); sets pair same-parity heads so each PSUM
# bank sees one PE row group only (mixed row groups in a bank = HW crash).
ESLOT = [0, 2, 1, 3, 4, 6, 5, 7]
SET_HEADS = [(0, 2), (1, 3), (4, 6), (5, 7)]


def _build():
    from contextlib import ExitStack, nullcontext

    import concourse.bass as bass
    import concourse.tile as tile
    from concourse import bacc, mybir
    from concourse.masks import make_identity

    f32 = mybir.dt.float32
    bf16 = mybir.dt.bfloat16

    nc = bacc.Bacc(
        "TRN2",
        target_bir_lowering=False,
        debug=False,
        enable_asserts=False,
        num_devices=NCORES,
    )
    kn_ap = nc.dram_tensor("key_n", [BL, S, C], bf16, kind="ExternalInput").ap()
    kt_ap = nc.dram_tensor("key_t", [BL, C, S], bf16, kind="ExternalInput").ap()
    qt_ap = nc.dram_tensor("q_t", [BL, C, N], bf16, kind="ExternalInput").ap()
    wq_ap = nc.dram_tensor("Wq_b", [C, C], bf16, kind="ExternalInput").ap()
    wk_ap = nc.dram_tensor("Wk_b", [C, C], bf16, kind="ExternalInput").ap()
    out_ap = nc.dram_tensor("out", [BL, N, C], f32, kind="ExternalOutput").ap()

    NT = S // 128          # 32 s-tiles of 128
    NJ = S // 512          # 8 macro windows of 512 rows
    NCK = C // 128         # 4 c_in chunks
    NM = C // 128          # 4 c_out chunks
    VW = HD + 2            # 66: per-head stride in vaug (64 v + 1 recip + 1 pad)

    with tile.TileContext(nc) as tc, ExitStack() as ctx:
        const = ctx.enter_context(tc.tile_pool(name="const", bufs=1))
        wq_bf = const.tile([128, NCK * C], bf16)
        wk_bf = const.tile([128, NCK * C], bf16)
        nc.sync.dma_start(
            wq_bf[:].rearrange("p (k c) -> p k c", k=NCK),
            wq_ap.rearrange("(k p) c -> p k c", k=NCK),
        )
        nc.sync.dma_start(
            wk_bf[:].rearrange("p (k c) -> p k c", k=NCK),
            wk_ap.rearrange("(k p) c -> p k c", k=NCK),
        )
        identf = const.tile([128, 128], f32)
        make_identity(nc, identf[:])

        # SBUF pools
        kb_pool = ctx.enter_context(tc.tile_pool(name="kb", bufs=2))
        ktt_pool = ctx.enter_context(tc.tile_pool(name="ktt", bufs=2))
        qpool = ctx.enter_context(tc.tile_pool(name="qpool", bufs=2))
        ktpj_pool = ctx.enter_context(tc.tile_pool(name="ktpj", bufs=2))
        epool = ctx.enter_context(tc.tile_pool(name="epool", bufs=4))
        spool = ctx.enter_context(tc.tile_pool(name="spool", bufs=4))
        fpool = ctx.enter_context(tc.tile_pool(name="fpool", bufs=2))
        vpool = ctx.enter_context(tc.tile_pool(name="vpool", bufs=3))
        opool = ctx.enter_context(tc.tile_pool(name="opool", bufs=2))
        oscpool = ctx.enter_context(tc.tile_pool(name="oscpool", bufs=1))

        # PSUM: sc0/sc1/sc2 1 bank each (score-set rotation + stage-D
        # transposes), kprj 1 bank (k/q projection accumulator), oacc 4.
        sc_pool = ctx.enter_context(tc.tile_pool(name="sc", bufs=1, space="PSUM"))
        kprj_pool = ctx.enter_context(tc.tile_pool(name="kprj", bufs=1, space="PSUM"))
        oacc_pool = ctx.enter_context(tc.tile_pool(name="oacc", bufs=1, space="PSUM"))

        def q_path(b):
            qts = qpool.tile([128, NCK * N], bf16, tag="qts")
            nc.sync.dma_start(
                qts[:].rearrange("p (k n) -> p k n", k=NCK),
                qt_ap[b].rearrange("(k p) n -> p k n", k=NCK),
            )
            qtp = qpool.tile([128, NM * N], bf16, tag="qtp")
            for m in range(NM):
                # sc2 bank, not kprj: lets qproj run concurrently with the
                # window-0 k-projection at batch boundaries
                pq = sc_pool.tile([128, 512], f32, tag="sc2")
                for k in range(NCK):
                    nc.tensor.matmul(
                        pq[:, :N],
                        lhsT=wq_bf[:, k * C + m * 128 : k * C + (m + 1) * 128],
                        rhs=qts[:, k * N : (k + 1) * N],
                        start=(k == 0),
                        stop=(k == NCK - 1),
                    )
                nc.vector.tensor_copy(qtp[:, m * N : (m + 1) * N], pq[:, :N])
            return qtp

        def load_window(b, kb, j):
            nc.sync.dma_start(
                kb[:, 4 * j * C : 4 * (j + 1) * C].rearrange(
                    "p (t c) -> p t c", t=4
                ),
                kn_ap[b, j * 512 : (j + 1) * 512, :].rearrange(
                    "(t p) c -> p t c", t=4
                ),
            )
            ktt = ktt_pool.tile([128, NCK * 512], bf16, tag="ktt")
            nc.sync.dma_start(
                ktt[:].rearrange("p (k s) -> p k s", k=NCK),
                kt_ap[b][:, j * 512 : (j + 1) * 512].rearrange(
                    "(k p) s -> p k s", k=NCK
                ),
            )
            return ktt

        def kproj_chunk(ktt, ktpj, j, m, use_scalar=False):
            pk = kprj_pool.tile([128, 512], f32, tag="kprj")
            for k in range(NCK):
                nc.tensor.matmul(
                    pk[:],
                    lhsT=wk_bf[:, k * C + m * 128 : k * C + (m + 1) * 128],
                    rhs=ktt[:, k * 512 : (k + 1) * 512],
                    start=(k == 0),
                    stop=(k == NCK - 1),
                )
            if use_scalar:
                nc.scalar.copy(
                    ktpj[:, m * S + j * 512 : m * S + (j + 1) * 512], pk[:]
                )
            else:
                nc.vector.tensor_copy(
                    ktpj[:, m * S + j * 512 : m * S + (j + 1) * 512], pk[:]
                )

        # state prepared for the NEXT batch during the current batch's tail
        nextb = {}

        def prep_batch(b, use_scalar_copies):
            qtp = q_path(b)
            kb = kb_pool.tile([128, NT * C], bf16, tag="kb")
            ktpj = ktpj_pool.tile([128, NM * S], bf16, tag="ktpj")
            ktt0 = load_window(b, kb, 0)
            return {"qtp": qtp, "kb": kb, "ktpj": ktpj, "ktt0": ktt0}

        for b in range(BL):
            st = prep_batch(b, True)
            for m in range(NM):
                kproj_chunk(st["ktt0"], st["ktpj"], 0, m,
                            use_scalar=(m % 2 == 0))
            qtp, kb, ktpj = st["qtp"], st["kb"], st["ktpj"]
            ktt_next = load_window(b, kb, 1)

            # scores: 1-bank sets (2 same-parity heads), strict 3-tag rotation
            def score_set(t, q):
                scp = sc_pool.tile([128, 2 * N], f32, tag=f"sc{(4 * t + q) % 3}")
                for pos in range(2):
                    h = SET_HEADS[q][pos]
                    m, hp = h // 2, (h % 2) * 64
                    nc.tensor.matmul(
                        scp[:, pos * N : (pos + 1) * N],
                        lhsT=ktpj[
                            hp : hp + 64, m * S + t * 128 : m * S + t * 128 + 128
                        ],
                        rhs=qtp[hp : hp + 64, m * N : (m + 1) * N],
                        start=True,
                        stop=True,
                    )
                return scp

            # ---------------- attention loop ----------------
            oacc = oacc_pool.tile([128, H * N], f32, tag="oacc")
            sets = {(0, q): score_set(0, q) for q in range(3)}
            pend = None  # et of tile t-1; den/vaug/out issued next iter

            def issue_out(t, et, vaug):
                crit = (
                    tc.tile_critical()
                    if (t == 0 or t == NT - 1)
                    else nullcontext()
                )
                with crit:
                    for h in range(H):
                        nc.tensor.matmul(
                            oacc[0:65, h * N : (h + 1) * N],
                            lhsT=vaug[:, h * VW : h * VW + HD + 1],
                            rhs=et[:, ESLOT[h] * N : (ESLOT[h] + 1) * N],
                            start=(t == 0 and h % 2 == 0),
                            stop=(t == NT - 1 and h % 2 == 1),
                            skip_group_check=True,
                        )

            def den_vaug_out(t, et):
                """den chain + vaug + out-MMs for tile t (inputs all ready)."""
                den = spool.tile([128, H], f32, tag="den")
                rcp = spool.tile([128, H], f32, tag="rcp")
                etf = fpool.tile([128, 8 * 128], bf16, tag="etf")
                etg = fpool.tile([128, 8 * 64], bf16, tag="etg")
                vaug = vpool.tile([128, H * VW], bf16, tag="vaug")
                vaug3 = vaug[:].rearrange("p (h c) -> p h c", c=VW)
                et3 = et[:].rearrange("p (h n) -> p h n", h=8)
                etf3 = etf[:].rearrange("p (h n) -> p h n", h=8)
                etg3 = etg[:].rearrange("p (h n) -> p h n", h=8)
                nc.vector.tensor_tensor(
                    etf3, et3[:, :, 0:128], et3[:, :, 128:256],
                    mybir.AluOpType.add,
                )
                nc.vector.tensor_tensor(
                    etg3, etf3[:, :, 0:64], etf3[:, :, 64:128],
                    mybir.AluOpType.add,
                )
                nc.vector.tensor_reduce(
                    den[:], etg3, mybir.AxisListType.X, mybir.AluOpType.add,
                )
                nc.vector.reciprocal(rcp[:], den[:])
                # div column (slot->head reorder = middle-swap per 4-group)
                nc.gpsimd.tensor_copy(
                    vaug3[:, :, HD : HD + 1].rearrange(
                        "p (g b a) c -> p g b a c", g=2, b=2
                    ),
                    rcp[:]
                    .rearrange("p (g a b) -> p g a b", g=2, a=2)
                    .transpose([0, 1, 3, 2])[:, :, :, :, None],
                )
                for half in range(2):
                    h0 = half * 4
                    nc.gpsimd.tensor_tensor(
                        vaug3[:, h0 : h0 + 4, 0:HD],
                        kb[:, t * C + h0 * HD : t * C + (h0 + 4) * HD].rearrange(
                            "p (h c) -> p h c", h=4
                        ),
                        vaug3[:, h0 : h0 + 4, HD : HD + 1].broadcast_to(
                            (128, 4, HD)
                        ),
                        mybir.AluOpType.mult,
                    )
                issue_out(t, et, vaug)

            for t in range(NT):
                et = epool.tile([128, H * N], bf16, tag="et")

                # 0) fused k-projection of window t//4+1, chunk t%4 (must
                #    precede the scores that read this window); DMA one
                #    window ahead.
                j = t // 4 + 1
                if j < NJ:
                    m = t % 4
                    kproj_chunk(ktt_next, ktpj, j, m, use_scalar=False)
                    if m == 3 and j + 1 < NJ:
                        ktt_next = load_window(b, kb, j + 1)

                # 1) exp for sets 0-2 (scored last iteration)
                for q in range(0, 3):
                    nc.scalar.activation(
                        et[:, q * 512 : (q + 1) * 512],
                        sets.pop((t, q))[:],
                        mybir.ActivationFunctionType.Exp,
                        scale=SCALE,
                    )
                # 2) this tile's set 3 + its act, issued BEFORE the previous
                #    tile's out-MMs so act3 is not gated on the vaug->out
                #    chain through the tensor FIFO.
                s3 = score_set(t, 3)
                nc.scalar.activation(
                    et[:, 3 * 512 : 4 * 512],
                    s3[:],
                    mybir.ActivationFunctionType.Exp,
                    scale=SCALE,
                )
                # 3) next tile's set 0 ahead of the out stream: the first
                #    act of each tile is the scalar pacer's longest wait.
                if t + 1 < NT:
                    sets[(t + 1, 0)] = score_set(t + 1, 0)
                # 4) everything for tile t-1 whose inputs are ready: den
                #    chain, vaug, out-MMs.
                if pend is not None:
                    den_vaug_out(t - 1, pend)
                # 5) next tile's sets 1-2
                if t + 1 < NT:
                    for q in (1, 2):
                        sets[(t + 1, q)] = score_set(t + 1, q)
                pend = et
            den_vaug_out(NT - 1, pend)

            # ---------------- epilogue ----------------
            # PSUM->SBUF once per head (scalar/DVE split), then per n-chunk:
            # transpose back (div rides as col 64) and scale by 1/max(div,1).
            oscs = []
            for half in range(2):
                osc = oscpool.tile([65, 4 * N], f32, tag=f"osc{half}")
                oscs.append(osc)
                for hh in range(4):
                    h = half * 4 + hh
                    if half == 0:
                        nc.scalar.copy(
                            osc[:, hh * N : (hh + 1) * N],
                            oacc[0:65, h * N : (h + 1) * N],
                        )
                    else:
                        nc.vector.tensor_copy(
                            osc[:, hh * N : (hh + 1) * N],
                            oacc[0:65, h * N : (h + 1) * N],
                        )
            for ncn in range(2):
                osb = opool.tile([128, C], f32, tag="osb")
                for half in range(2):
                    tp = sc_pool.tile([128, 512], f32, tag=f"sc{half}")
                    for hh in range(4):
                        nc.tensor.transpose(
                            tp[:, hh * 65 : hh * 65 + 65],
                            oscs[half][
                                0:65, hh * N + ncn * 128 : hh * N + ncn * 128 + 128
                            ],
                            identf[0:65, 0:65],
                        )
                    for hh in range(4):
                        h = half * 4 + hh
                        dm = spool.tile([128, 2], f32, tag="dm")
                        nc.vector.tensor_scalar_max(
                            dm[:, 0:1], tp[:, hh * 65 + 64 : hh * 65 + 65], 1.0
                        )
                        nc.vector.reciprocal(dm[:, 1:2], dm[:, 0:1])
                        nc.vector.tensor_scalar_mul(
                            osb[:, h * HD : (h + 1) * HD],
                            tp[:, hh * 65 : hh * 65 + HD],
                            dm[:, 1:2],
                        )
                nc.sync.dma_start(out_ap[b, ncn * 128 : (ncn + 1) * 128, :], osb[:])

    nc.compile()
    return nc


def _get_nc():
    with _lock:
        if "nc" not in _cache:
            _cache["nc"] = _build()
        return _cache["nc"]


def _prep_core_inputs(query, key, Wq, Wk):
    import ml_dtypes

    bf = ml_dtypes.bfloat16
    key_n = np.ascontiguousarray(key.astype(bf))                      # [B, S, C]
    key_t = np.ascontiguousarray(key.transpose(0, 2, 1).astype(bf))   # [B, C, S]
    q_t = np.ascontiguousarray(query.transpose(0, 2, 1).astype(bf))   # [B, C, N]
    wq_b = np.ascontiguousarray(Wq.astype(bf))
    wk_b = np.ascontiguousarray(Wk.astype(bf))
    return [
        {
            "key_n": key_n[c * BL : (c + 1) * BL],
            "key_t": key_t[c * BL : (c + 1) * BL],
            "q_t": q_t[c * BL : (c + 1) * BL],
            "Wq_b": wq_b,
            "Wk_b": wk_b,
        }
        for c in range(NCORES)
    ]


def kernel(query, key, Wq, Wk):
    from concourse.bass_utils import run_bass_kernel_spmd

    nc = _get_nc()
    query = np.ascontiguousarray(query, dtype=np.float32)
    key = np.ascontiguousarray(key, dtype=np.float32)
    Wq = np.ascontiguousarray(Wq, dtype=np.float32)
    Wk = np.ascontiguousarray(Wk, dtype=np.float32)
    in_maps = _prep_core_inputs(query, key, Wq, Wk)
    res = run_bass_kernel_spmd(nc, in_maps, core_ids=list(range(NCORES)))
    out = np.concatenate([r["out"] for r in res.results], axis=0)
    return out, out
